# revision 49
# baseline (speedup 1.0000x reference)
"""Multi-head self-attention (B=4, S=2048, D=1024, 16 heads x 64) on 8 TRN2
NeuronCores via Bass/Tile.

Sharding: tensor-parallel over heads. Each core owns 2 heads (128 of the 1024
Q/K/V output features, column-parallel) and the matching 128 rows of Wo
(row-parallel). Every core computes a full-shape partial output (bf16); the
host sums the 8 partials (the row-parallel all-reduce) after gathering.

v2: explicit ACT-paced software pipeline. The kernel is triple-roofline-bound
(PE ~283us of matmul slots, ACT ~285us of exp streaming, DVE ~200us of PSUM
drains), so the schedule keeps the scalar engine 100% busy through the
attention phase and fills the PE slack in each exp window with next-batch QKV
and prev-batch out-projection matmuls.

Per-core dataflow (matmul operands bf16, accumulation fp32 in PSUM):
  xT[b]   : [D, S] features-on-partitions (host pre-transposed)
  qT/kT/vT: [128, S]  = (x @ W)^T per core, via lhsT=W k-tiles, rhs=xT
  v_aug   : PE-transpose of vT -> v natural [S,64] per head + ones column
  scoresT : per (ich, jt): two row-group-concurrent K=64 matmuls (the two
            heads at PE rows 0-63 / 64-127) -> one [128,1024] PSUM tile
  exp     : one ACT Exp op per (ich, jt) covering both heads (scale=1/8,
            per-partition bias = attention-mask column) -> et bf16 SBUF
  PV      : lhsT=[v_h | ones] [128 j, 65], rhs=expT half, accumulated over
            j-tiles -> rows 0-63 ctx^T, row 64 = softmax denominator
  norm    : drain pc to SBUF (frees PSUM fast), reciprocal_approx_fast of the
            denom row, GPSIMD partition broadcast, DVE multiply -> ctxT bf16
  out     : lhsT=ctxT tile [128,128], rhs=Wo_c [128,512]; DVE adds bo/8 while
            draining PSUM->SBUF bf16; DMA bf16 partial to DRAM

Pipeline (per batch, 64 windows of one (ich, jt) each): window w emits
scores(w) [PE], exp(w) [ACT], PV(w-2) [PE], plus ~2 slots of filler pulled
from a shared pool of QKV(b') / out-projection(b') units. sc PSUM tiles are
double-buffered so exp(w-1) runs while scores(w) is written; PV lags two
windows so et is always ready.
"""

import numpy as np
import ml_dtypes

import concourse.bass as bass
import concourse.mybir as mybir
import concourse.tile as tile
from concourse import bacc, bass_utils
from concourse.masks import make_identity

F32 = mybir.dt.float32
BF16 = mybir.dt.bfloat16
AF = mybir.ActivationFunctionType
BF = ml_dtypes.bfloat16
ts = bass.ts

B, S, D = 4, 2048, 1024
NH, HD = 16, 64
NCORES = 8
OF = D // NCORES            # 128 out-features per core (2 heads)
NKT = D // 128              # 8 contraction tiles
NJT = S // 128              # 16 key tiles per batch
NICH = S // 512             # 4 query chunks per batch
NTT = S // 128              # 16 token tiles per batch
NW = NICH * NJT             # 64 attention windows per batch
PVLAG = 2                   # PV trails scores by 2 windows (et ready)


def build_program():
    nc = bacc.Bacc("TRN2", target_bir_lowering=False, debug=False,
                   num_devices=NCORES)
    xT_d = nc.dram_tensor("xT", [B, D, S], BF16, kind="ExternalInput")
    # host pre-arranges W into [128, NKT, OF] (partition-major) so the
    # weight DMA is contiguous instead of 256B-strided
    wq_d = nc.dram_tensor("wq", [128, NKT, OF], BF16, kind="ExternalInput")
    wk_d = nc.dram_tensor("wk", [128, NKT, OF], BF16, kind="ExternalInput")
    wv_d = nc.dram_tensor("wv", [128, NKT, OF], BF16, kind="ExternalInput")
    bq_d = nc.dram_tensor("bq", [OF, 1], F32, kind="ExternalInput")
    bk_d = nc.dram_tensor("bk", [OF, 1], F32, kind="ExternalInput")
    bv_d = nc.dram_tensor("bv", [OF, 1], F32, kind="ExternalInput")
    wo_d = nc.dram_tensor("wo", [OF, D], BF16, kind="ExternalInput")
    mask_d = nc.dram_tensor("mask", [128, B * NJT], F32, kind="ExternalInput")
    out_d = nc.dram_tensor("out", [B * S, D], BF16, kind="ExternalOutput")

    with tile.TileContext(nc) as tc:
        with (
            tc.tile_pool(name="consts", bufs=1) as consts,
            tc.tile_pool(name="xin", bufs=3) as xin,
            tc.tile_pool(name="qkv", bufs=2) as qkv,
            tc.tile_pool(name="attn", bufs=6) as attn,
            tc.tile_pool(name="ctxp", bufs=2) as ctxp,
            tc.tile_pool(name="outp", bufs=4) as outp,
            tc.tile_pool(name="psum", bufs=2, space="PSUM") as psum,
        ):
            # ---------------- constants ----------------
            # qkv weights go first on the sync DMA queue (needed by MM #1);
            # wo/mask ride the scalar-engine DMA queue so the first x chunk
            # isn't stuck behind them
            w_sb = {}
            for nm, d in (("q", wq_d), ("k", wk_d), ("v", wv_d)):
                t = consts.tile([128, NKT, OF], BF16, name=f"w{nm}_sb")
                nc.sync.dma_start(t, d[:, :, :])
                w_sb[nm] = t
            b_sb = {}
            for nm, d in (("q", bq_d), ("k", bk_d), ("v", bv_d)):
                t = consts.tile([OF, 1], F32, name=f"b{nm}_sb")
                nc.sync.dma_start(t, d[:, :])
                b_sb[nm] = t
            ident = consts.tile([128, 128], BF16)
            make_identity(nc, ident)
            wo_sb = consts.tile([OF, D], BF16)
            nc.scalar.dma_start(wo_sb, wo_d[:, :])
            mask_sb = consts.tile([128, B * NJT], F32)
            nc.scalar.dma_start(mask_sb, mask_d[:, :])

            state = [dict(q_chunks=0, kv_chunks=0, ctx_ready=0,
                          ready_win=[0] * NICH)
                     for _ in range(B)]
            clock = dict(win=0)  # global window counter (emission time)

            # ---------------- unit generators ----------------
            def qkv_units(b):
                """QKV projections + V transpose for batch b. One yield per
                PE slot (~216ns). Batch 0 runs KV for all chunks first so
                attention(0, ich0) can sweep all 16 j-tiles without the
                per-chunk staircase; q chunks 1-3 then fill in under the
                ich0 windows."""
                st = state[b]
                pT = {nm: qkv.tile([OF, S], BF16, name=f"{nm}T")
                      for nm in ("q", "k", "v")}
                st["pT"] = pT
                va = [qkv.tile([128, NJT, 65], BF16, name=f"v_aug{h}")
                      for h in range(2)]
                st["va"] = va
                for h in range(2):
                    nc.vector.memset(va[h][:, :, 64:65], 1.0)
                xts = {}

                def chunk_dma(nch):
                    # two half-chunk DMAs so the first k-tile matmuls can
                    # start as soon as half the chunk has landed
                    xt = xin.tile([128, NKT, 512], BF16, name="xt", bufs=8)
                    src = xT_d[b].rearrange("(k p) t -> p k t", p=128)[
                        :, :, ts(nch, 512)]
                    nc.sync.dma_start(xt[:, 0:4, :], src[:, 0:4, :])
                    nc.sync.dma_start(xt[:, 4:8, :], src[:, 4:8, :])
                    xts[nch] = xt

                def proj(nm, nch):
                    ps = psum.tile([128, 512], F32, tag="mm", name="ps_qkv")
                    for kt in range(NKT):
                        nc.tensor.matmul(
                            ps, lhsT=w_sb[nm][:, kt, :],
                            rhs=xts[nch][:, kt, :],
                            start=(kt == 0), stop=(kt == NKT - 1),
                        )
                        yield
                    nc.vector.tensor_scalar_add(
                        pT[nm][:, ts(nch, 512)], ps, b_sb[nm])

                def transposes(nch):
                    for jt in range(4 * nch, 4 * nch + 4):
                        pvt = psum.tile([128, 128], BF16, tag="mm",
                                        name="pvt")
                        nc.tensor.transpose(
                            pvt, pT["v"][:, ts(jt, 128)], ident)
                        for h in range(2):
                            nc.vector.tensor_copy(
                                va[h][:, jt, 0:64],
                                pvt[:, h * 64:(h + 1) * 64])
                        yield

                for nch in range(NICH):
                    chunk_dma(nch)
                if b == 0:
                    for nm in ("q", "k", "v"):
                        yield from proj(nm, 0)
                    yield from transposes(0)
                    st["q_chunks"] = st["kv_chunks"] = 1
                    for nch in range(1, NICH):
                        for nm in ("k", "v"):
                            yield from proj(nm, nch)
                        yield from transposes(nch)
                        st["kv_chunks"] = nch + 1
                    for nch in range(1, NICH):
                        yield from proj("q", nch)
                        st["q_chunks"] = nch + 1
                else:
                    for nch in range(NICH):
                        for nm in ("q", "k", "v"):
                            yield from proj(nm, nch)
                        yield from transposes(nch)
                        st["q_chunks"] = st["kv_chunks"] = nch + 1

            tail_mode = dict(on=False, n=0)

            def outproj_units(b):
                """Output projection for batch b; one yield per (token tile,
                512-feature half). Gated per-ich on state[b]['ctx_ready'].
                In the kernel tail (ACT idle) the PSUM drains alternate
                between DVE and ScalarE."""
                st = state[b]
                ctxT = st["ctxT"]
                for ich in range(NICH):
                    # wait until the norm chain (recip/broadcast/mul) has had
                    # ~3 windows to finish, else the po matmul chains the PE
                    # stream behind that latency
                    while (st["ctx_ready"] <= ich
                           or clock["win"] < st["ready_win"][ich] + 3):
                        yield False  # not ready; weaver retries later
                    for tt in range(4 * ich, 4 * ich + 4):
                        for oc in range(2):
                            po = psum.tile([128, 512], F32, tag="mm",
                                           name="po")
                            nc.tensor.matmul(
                                po, lhsT=ctxT[:, ts(tt, 128)],
                                rhs=wo_sb[:, ts(oc, 512)],
                                start=True, stop=True,
                            )
                            # bo is added host-side after the partial-sum
                            # gather, so the drain is a pure cast-copy
                            osb = outp.tile([128, 512], BF16, name="osb")
                            tail_mode["n"] += 1
                            if tail_mode["on"] and tail_mode["n"] % 2:
                                nc.scalar.copy(osb, po)
                            else:
                                nc.vector.tensor_copy(osb, po)
                            nc.sync.dma_start(
                                out_d[b * S + tt * 128:
                                      b * S + (tt + 1) * 128, ts(oc, 512)],
                                osb)
                            yield True

            def attention_windows(b):
                """Attention for batch b as NW+PVLAG windows. Each next()
                emits: scores-pair(w) [PE] + exp(w) [ACT] + PV(w-PVLAG) [PE].
                Yields the window's chunk requirement before emitting."""
                st = state[b]
                qT, kT = st["pT"]["q"], st["pT"]["k"]
                va = st["va"]
                ctxT = ctxp.tile([128, S], BF16, name="ctxT")
                st["ctxT"] = ctxT
                pc = {}
                ets = {}

                def emit_scores(w):
                    ich, jt = divmod(w, NJT)
                    isl = ts(ich, 512)
                    if jt == 0:
                        pc[ich % 2] = [
                            psum.tile([128, 512], F32, tag="pc",
                                      name=f"pc{h}")
                            for h in range(2)]
                    sc = psum.tile([128, 1024], F32, tag="sc", name="sc")
                    for h in range(2):
                        hs = slice(h * 64, (h + 1) * 64)
                        nc.tensor.matmul(
                            sc[:, ts(h, 512)],
                            lhsT=kT[hs, ts(jt, 128)], rhs=qT[hs, isl],
                            start=True, stop=True,
                        )
                    return sc

                def emit_exp(w, sc):
                    et = attn.tile([128, 1024], BF16, name="et", bufs=6)
                    col = b * NJT + w % NJT
                    nc.scalar.activation(
                        et, sc, AF.Exp,
                        bias=mask_sb[:, col:col + 1], scale=0.125)
                    ets[w] = et

                def emit_pv(pw):
                    pich, pjt = divmod(pw, NJT)
                    pcs = pc[pich % 2]
                    pet = ets.pop(pw)
                    for h in range(2):
                        nc.tensor.matmul(
                            pcs[h][0:65, :], lhsT=va[h][:, pjt, :],
                            rhs=pet[:, ts(h, 512)],
                            start=(pjt == 0), stop=(pjt == NJT - 1),
                        )
                    if pjt == NJT - 1:
                        # normalize: den copies + recips first so the gpsimd
                        # broadcasts overlap the DVE stream; the muls read
                        # pc directly from PSUM (the next ich's first PV is
                        # 2 windows out — pc frees in time)
                        pisl = ts(pich, 512)
                        den, rec, rep = {}, {}, {}
                        for h in range(2):
                            den[h] = attn.tile([1, 512], F32,
                                               name=f"den{h}", bufs=2)
                            nc.vector.tensor_copy(den[h], pcs[h][64:65, :])
                            rec[h] = attn.tile([1, 512], F32,
                                               name=f"rec{h}", bufs=2)
                            nc.vector.reciprocal_approx_fast(rec[h], den[h])
                            rep[h] = attn.tile([64, 512], F32,
                                               name=f"rep{h}", bufs=2)
                            nc.gpsimd.partition_broadcast(rep[h], rec[h])
                        for h in range(2):
                            nc.vector.tensor_mul(
                                ctxT[h * 64:(h + 1) * 64, pisl],
                                pcs[h][0:64, :], rep[h])
                        st["ready_win"][pich] = clock["win"]
                        st["ctx_ready"] = pich + 1

                # group pipeline: group g emits scores+exp for windows
                # (2g, 2g+1) back-to-back — the four row-group-alternating
                # score matmuls keep the LDWEIGHTS pull-ahead unblocked (no
                # K=128 matmul between) — then the PVs for the previous
                # group's windows.
                NG = NW // 2
                for g in range(NG + 1):
                    if g < NG:
                        w0 = 2 * g
                        ich, jt = divmod(w0 + 1, NJT)
                        while (st["q_chunks"] <= ich
                               or st["kv_chunks"] <= jt // 4):
                            yield "stall"  # weaver must advance qkv first
                        sc0 = emit_scores(w0)
                        sc1 = emit_scores(w0 + 1)
                        emit_exp(w0, sc0)
                        emit_exp(w0 + 1, sc1)
                    if g >= 1:
                        emit_pv(2 * (g - 1))
                        emit_pv(2 * (g - 1) + 1)
                    clock["win"] += 1
                    yield "group"

            # ---------------- the weaver ----------------
            # group pace is PE-bound: 8 attention MMs (~1730ns) + filler;
            # ACT needs 2x~1005ns per group and fits underneath
            QKV_COST = 216       # one K=128 N=512 matmul
            OUT_COST = 450       # one N=512 matmul + 658ns DVE debt
            FILLER_BUDGET = 900  # per group

            def weave():
                g_qkv = [qkv_units(b) for b in range(B)]
                g_out = [outproj_units(b) for b in range(B)]
                ags = [None] * B
                qkv_done = [False] * B
                out_done = [False] * B
                pumped = [0] * B

                def pull_qkv(b):
                    if b >= B or qkv_done[b]:
                        return False
                    try:
                        next(g_qkv[b])
                        return True
                    except StopIteration:
                        qkv_done[b] = True
                        return False

                def pull_out():
                    """One outproj unit from the oldest batch with work."""
                    for bo in range(B):
                        if out_done[bo] or state[bo].get("ctxT") is None:
                            continue
                        if state[bo]["ctx_ready"] == 0:
                            continue
                        try:
                            r = g_out[bo].send(None)
                            if r is False:
                                continue  # gated; try younger batch
                            return True
                        except StopIteration:
                            out_done[bo] = True
                            continue
                    return False

                def fillers(b_att):
                    # exactly one outproj drain per group (paces its DVE
                    # debt evenly through the kernel instead of dumping all
                    # 128 units on batch 3 and the tail), then own-batch
                    # qkv (its windows are data-gated on it), then the next
                    # batch's qkv
                    budget = FILLER_BUDGET
                    if pull_out():
                        budget -= OUT_COST
                    while budget > 0:
                        if pull_qkv(b_att):
                            budget -= QKV_COST
                        elif pull_qkv(b_att + 1):
                            budget -= QKV_COST
                        elif budget >= OUT_COST and pull_out():
                            budget -= OUT_COST
                        else:
                            break

                def pump(b):
                    """Advance ags[b] by one window (absorbing qkv stalls)."""
                    stalls = 0
                    while True:
                        r = next(ags[b])
                        if r == "stall":
                            pull_qkv(b)
                            stalls += 1
                            if stalls > 10000:
                                raise RuntimeError("qkv starvation")
                            continue
                        pumped[b] += 1
                        return

                # prologue: batch 0 QKV chunk 0 dense
                while state[0]["q_chunks"] < 1:
                    pull_qkv(0)

                NG = NW // 2
                for b in range(B):
                    if ags[b] is None:
                        ags[b] = attention_windows(b)
                    while pumped[b] < NG:
                        pump(b)
                        fillers(b)
                    # tail: interleave this batch's PV-only tail group with
                    # the next batch's scores-only head group so ACT never
                    # bubbles across the batch boundary
                    if b + 1 < B:
                        ags[b + 1] = attention_windows(b + 1)
                        pump(b + 1)   # scores+exp head group
                        pump(b)       # PV tail group
                        fillers(b)
                    else:
                        pump(b)
                        fillers(b)
                # drain all remaining work
                for b in range(B):
                    while pull_qkv(b):
                        pass
                guard = 0
                tail_mode["on"] = True
                while not all(out_done):
                    clock["win"] += 1  # release ready_win gating in the tail
                    pull_out()
                    guard += 1
                    if guard > 10000:
                        raise RuntimeError("outproj drain stuck")

            weave()
    nc.finalize()
    return nc


def make_in_maps(x, attention_mask, Wq, bq, Wk, bk, Wv, bv, Wo, bo):
    x = np.asarray(x, dtype=np.float32)
    attention_mask = np.asarray(attention_mask, dtype=np.float32)
    Wq, Wk, Wv, Wo = (np.asarray(a, dtype=np.float32) for a in (Wq, Wk, Wv, Wo))
    bq, bk, bv, bo = (np.asarray(a, dtype=np.float32) for a in (bq, bk, bv, bo))

    xT = np.ascontiguousarray(x.transpose(0, 2, 1)).astype(BF)  # [B, D, S]
    # mask[b,0,0,j] -> [128 partitions, B*NJT] column per (batch, j-tile)
    m = attention_mask.reshape(B, S).reshape(B, NJT, 128)
    mask_host = np.ascontiguousarray(m.transpose(2, 0, 1).reshape(128, B * NJT))

    def w_arr(W, cs):
        # [D, OF] -> [128 partitions, NKT, OF] partition-major (contiguous
        # DMA into the SBUF tile layout)
        w = np.ascontiguousarray(W[:, cs]).astype(BF)
        return np.ascontiguousarray(
            w.reshape(NKT, 128, OF).transpose(1, 0, 2))

    in_maps = []
    for c in range(NCORES):
        cs = slice(c * OF, (c + 1) * OF)
        in_maps.append({
            "xT": xT,
            "wq": w_arr(Wq, cs),
            "wk": w_arr(Wk, cs),
            "wv": w_arr(Wv, cs),
            "bq": np.ascontiguousarray(bq[cs]).reshape(OF, 1),
            "bk": np.ascontiguousarray(bk[cs]).reshape(OF, 1),
            "bv": np.ascontiguousarray(bv[cs]).reshape(OF, 1),
            "wo": np.ascontiguousarray(Wo[cs, :]).astype(BF),
            "mask": mask_host,
        })
    return in_maps


def combine_outputs(results, bo):
    acc = np.zeros((B * S, D), dtype=np.float32)
    for r in results:
        acc += r["out"].astype(np.float32)
    acc += np.asarray(bo, dtype=np.float32)
    return acc.reshape(B, S, D)


_NC_CACHE = []


def _get_program():
    if not _NC_CACHE:
        _NC_CACHE.append(build_program())
    return _NC_CACHE[0]


def kernel(**inputs):
    nc = _get_program()
    in_maps = make_in_maps(**inputs)
    res = bass_utils.run_bass_kernel_spmd(
        nc, in_maps, core_ids=list(range(NCORES)))
    return combine_outputs(res.results, inputs["bo"])
